# revision 19
# baseline (speedup 1.0000x reference)
"""Bahdanau-style additive attention on 8 TRN2 NeuronCores.

  hidden = tanh(q @ Wq + k @ Wk)        (B, L, H)
  scores = hidden @ v_param             (B, L)
  attn   = softmax(scores, axis=-1)
  out    = attn @ v                     (B, D)

Sharding: data-parallel over batch — 4 batches per core (B=32, 8 cores).

v2 pipeline (all-bf16, fully SBUF-resident):
  - k (16 tiles of [128, 2048]) + v (4 slabs of [128, 64*DV]) all fit in
    SBUF (~130 KiB/partition of 208); every DMA is issued at t=0 so the
    DMA bus runs flat-out from the start (k on the SP HWDGE queue, v on
    the gpsimd SWDGE queue).
  - W1  pre[H, 1024] = Wk.T @ kT   two 512-col matmuls into one 2-bank
        PSUM tile (A/B alternate per group)
  - ACT tanh over the full 1024-col group (halves per-call PSUM access
        overhead vs 512)
  - W2  scores: stationary = hidden 128-col sub-chunks, moving = vp
        column -> score columns land in [128, 32] scolA/B (even/odd subs
        alternate banks so per-stop readouts overlap)
  - ACT exp once per batch per bank: [128, 32] -> strided w columns
  - W3  acc[1, DV] += w_col.T @ v_slab[:, tp*DV:...]  64 accumulating
        matmuls per batch; ones column in v gives the softmax
        denominator for free
  - host: out = acc[:D] / acc[D]
"""

import ml_dtypes
import numpy as np

import concourse.bass as bass
import concourse.mybir as mybir
from concourse.tile import TileContext

B, L, D, H = 32, 8192, 128, 128
NCORES = 8
BPC = B // NCORES  # batches per core
GRP = 1024  # L positions per W1/tanh group (2 psum banks)
NG = L // GRP  # 8 groups per batch
KTILE = 2048  # L positions per kT DMA tile
SUB = 128  # L positions per W2/W3 sub-chunk (stationary width)
NSUB = L // SUB  # 64 per batch
DV = 129  # v row: 128 data + ones col

F32 = mybir.dt.float32
BF16 = mybir.dt.bfloat16
ACTF = mybir.ActivationFunctionType

_CACHE = {}


def _split_excess_waits(nc, max_waits=1):
    """walrus in this env accepts at most one sync-wait per instruction;
    move extras onto InstNoOps placed just before (same engine, in order)."""
    for fn in nc.m.functions:
        for bb in fn.blocks:
            insts = list(bb.instructions)
            new_insts = []
            for ins in insts:
                si = ins.sync_info
                waits = list(si.on_wait) if si and si.on_wait else []
                if len(waits) > max_waits:
                    extra, keep = waits[:-max_waits], waits[-max_waits:]
                    for g0 in range(0, len(extra), max_waits):
                        pre = mybir.InstNoOp(
                            name=f"{ins.name}-waitsplit{g0}",
                            engine=ins.engine,
                            ins=[],
                            outs=[],
                            sync_info=mybir.SyncInfo(
                                on_wait=extra[g0 : g0 + max_waits], on_update=[]
                            ),
                        )
                        nc.register_instruction(pre, overwrite=True)
                        new_insts.append(pre)
                    ins.sync_info = mybir.SyncInfo(
                        on_wait=keep, on_update=list(si.on_update or [])
                    )
                new_insts.append(ins)
            if len(new_insts) != len(insts):
                bb.instructions[:] = new_insts


def build_nc_v2():
    nc = bass.Bass("TRN2")
    kb_in = nc.dram_tensor("kb", [BPC, D, L], BF16, kind="ExternalInput")
    # packed consts: cols 0:4 qwq (f32), 4:68 wk (128 bf16 cols), 68 vp pair
    cst_in = nc.dram_tensor("cst", [128, 69], F32, kind="ExternalInput")
    v_in = nc.dram_tensor("vv", [BPC, SUB, NSUB * DV], BF16, kind="ExternalInput")
    out_d = nc.dram_tensor("out", [1, BPC * DV], F32, kind="ExternalOutput")

    NKT = L // KTILE  # 4 k tiles per batch
    GSUB = GRP // SUB  # 8 sub-chunks per group

    with TileContext(nc) as tc:
        with (
            tc.tile_pool(name="const", bufs=1) as cpool,
            tc.tile_pool(name="kp", bufs=BPC * NKT) as kpool,
            tc.tile_pool(name="vp_", bufs=BPC) as vpool,
            tc.tile_pool(name="hp", bufs=5) as hpool,
            tc.tile_pool(name="wp", bufs=2) as wpool,
            tc.tile_pool(name="ob", bufs=1) as opool,
            tc.tile_pool(name="pre", bufs=1, space="PSUM") as pre_pool,
            tc.tile_pool(name="sps", bufs=1, space="PSUM") as s_pool,
            tc.tile_pool(name="ops", bufs=2, space="PSUM") as o_pool,
        ):
            # HAM warm-up on zeroed tiles: ramps the PE p-state while the
            # first k tiles stream in. Uses the preB tag so W1(g0) on preA
            # never waits for it.
            zwarm = cpool.tile([128, 512], BF16)
            nc.gpsimd.memset(zwarm[:], 0.0)
            warm_ps = pre_pool.tile([H, GRP], F32, tag="preB")
            for _ in range(8):
                nc.tensor.matmul(
                    warm_ps[:, :512], zwarm[:, :128], zwarm[:], start=True, stop=True
                )

            cst = cpool.tile([128, 69], F32)
            nc.sync.dma_start(cst[:], cst_in[:])
            qwq = cst[:, 0:4]
            wk = cst[:, 4:68].bitcast(BF16)
            vph = cst[:, 68:69].bitcast(BF16)[:, 0:1]

            out_sb = opool.tile([1, BPC * DV], F32)

            # ALL input DMAs on the one sync HWDGE queue, in consumption
            # order (k0, v0, k1, v1, ...): the queue is processed in order
            # and stripes each transfer across all 16 DMA engines, so
            # arrival order matches the pipeline's needs at full bus rate.
            kts = {}
            v_tiles = {}
            for b in range(BPC):
                for t in range(NKT):
                    ktile = kpool.tile([D, KTILE], BF16, tag="kt")
                    nc.sync.dma_start(
                        ktile[:], kb_in[b, :, t * KTILE : (t + 1) * KTILE]
                    )
                    kts[b, t] = ktile
                vtile = vpool.tile([SUB, NSUB * DV], BF16, tag="vt")
                nc.sync.dma_start(vtile[:], v_in[b])
                v_tiles[b] = vtile

            # 4-deep software pipeline over global group index s:
            #   segment s: W1(s) | tanh(s) | W2(s-2) | exp(s-2) | W3(s-3)
            # W2 trails tanh by TWO segments so the in-order PE queue
            # never waits on ACT; exp is emitted before tanh on ACT so the
            # scol bank frees early.
            NSEG = BPC * NG
            hhs = {}
            ws = {}
            accs = {}
            pend_exp = {}
            for s in range(NSEG + 3):
                # exp for segment s-2 first on ACT so its scol frees before
                # tanh(s) occupies the engine
                if s in pend_exp:
                    b2p, g2p, scolp = pend_exp.pop(s)
                    nc.scalar.activation(
                        ws[b2p][:, g2p * GSUB : (g2p + 1) * GSUB], scolp[:], ACTF.Exp
                    )
                if s < NSEG:
                    b1, g1 = divmod(s, NG)
                    ktile = kts[b1, g1 // (KTILE // GRP)]
                    koff = (g1 % (KTILE // GRP)) * GRP
                    pre = pre_pool.tile(
                        [H, GRP], F32, tag="preA" if s % 2 == 0 else "preB"
                    )
                    for c in range(2):
                        nc.tensor.matmul(
                            pre[:, c * 512 : (c + 1) * 512],
                            wk[:],
                            ktile[:, koff + c * 512 : koff + (c + 1) * 512],
                            start=True,
                            stop=True,
                        )
                    hh = hpool.tile([H, GRP], BF16, tag="hh")
                    nc.scalar.activation(
                        hh[:], pre[:], ACTF.Tanh, bias=qwq[:, b1 : b1 + 1], scale=1.0
                    )
                    hhs[s] = hh
                if 2 <= s <= NSEG + 1:
                    b2, g2 = divmod(s - 2, NG)
                    if g2 == 0:
                        ws[b2] = wpool.tile([SUB, NSUB], BF16, tag="w", name=f"w{b2}")
                    hh = hhs.pop(s - 2)
                    scol = s_pool.tile(
                        [SUB, GSUB], F32, tag="scolA" if g2 % 2 == 0 else "scolB"
                    )
                    for j in range(GSUB):
                        nc.tensor.matmul(
                            scol[:, j : j + 1],
                            hh[:, j * SUB : (j + 1) * SUB],
                            vph[:],
                            start=True,
                            stop=True,
                        )
                    pend_exp[s + 1] = (b2, g2, scol)
                if 3 <= s <= NSEG + 2:
                    b3, g3 = divmod(s - 3, NG)
                    if g3 == 0:
                        accs[b3] = o_pool.tile([1, DV], F32, tag="acc", name=f"acc{b3}")
                    vtile = v_tiles[b3]
                    w = ws[b3]
                    for j in range(GSUB):
                        tp = g3 * GSUB + j
                        nc.tensor.matmul(
                            accs[b3][:],
                            w[:, tp : tp + 1],
                            vtile[:, tp * DV : (tp + 1) * DV],
                            start=(tp == 0),
                            stop=(tp == NSUB - 1),
                        )
                    if g3 == NG - 1:
                        # copy on the otherwise-idle DVE: on ACT this copy
                        # waits for PE's last W3 and blocks the next tanh
                        nc.vector.tensor_copy(
                            out_sb[:, b3 * DV : (b3 + 1) * DV], accs[b3][:]
                        )

            nc.sync.dma_start(out_d[:], out_sb[:])

    _split_excess_waits(nc)
    return nc


def _prep_inputs(q, k, v, W_line, v_param):
    """Host-side shard + layout prep. Returns per-core input maps."""
    qWq = q.astype(np.float64) @ W_line[:D].astype(np.float64)  # (B, H)
    wk = np.ascontiguousarray(W_line[D:]).astype(np.float32)  # (D, H)
    wkb = np.ascontiguousarray(wk.astype(ml_dtypes.bfloat16))
    vpb = v_param.astype(ml_dtypes.bfloat16)
    vpair = np.ascontiguousarray(
        np.stack([vpb, np.zeros_like(vpb)], axis=1)
    )  # [H, 2] bf16 -> one f32 col

    in_maps = []
    for c in range(NCORES):
        bs = slice(c * BPC, (c + 1) * BPC)
        kT = np.ascontiguousarray(k[bs].transpose(0, 2, 1))  # (BPC, D, L)
        vv = np.zeros((BPC, L, DV), dtype=np.float32)
        vv[:, :, :D] = v[bs]
        vv[:, :, D] = 1.0
        # SBUF slab layout per batch: [p=l%128, (l//128)*DV + d]
        vv = np.ascontiguousarray(
            vv.reshape(BPC, NSUB, SUB, DV).transpose(0, 2, 1, 3).reshape(
                BPC, SUB, NSUB * DV
            )
        )
        qwq = np.ascontiguousarray(qWq[bs].T.astype(np.float32))  # (H, BPC)
        cst = np.zeros((128, 69), dtype=np.float32)
        cst[:, 0:4] = qwq
        cst[:, 4:68] = wkb.view(np.float32)
        cst[:, 68:69] = vpair.view(np.float32)
        in_maps.append(
            {
                "kb": kT.astype(ml_dtypes.bfloat16),
                "vv": vv.astype(ml_dtypes.bfloat16),
                "cst": cst,
            }
        )
    return in_maps


def _gather_output(results):
    out = np.empty((B, D), dtype=np.float32)
    for c, r in enumerate(results):
        rows = r["out"].reshape(BPC, DV).astype(np.float64)
        out[c * BPC : (c + 1) * BPC] = (rows[:, :D] / rows[:, D : D + 1]).astype(
            np.float32
        )
    return out


def run(q, k, v, W_line, v_param, trace=False, **spmd_kwargs):
    from concourse.bass_utils import run_bass_kernel_spmd

    if "nc" not in _CACHE:
        _CACHE["nc"] = build_nc_v2()
    nc = _CACHE["nc"]
    in_maps = _prep_inputs(q, k, v, W_line, v_param)
    res = run_bass_kernel_spmd(
        nc, in_maps, list(range(NCORES)), trace=trace, **spmd_kwargs
    )
    return _gather_output(res.results), res


def kernel(q, k, v, W_line, v_param):
    out, _ = run(q, k, v, W_line, v_param, trace=False)
    return out
